# revision 8
# baseline (speedup 1.0000x reference)
"""Paged-attention GQA decode kernel for Trainium2 (8 NeuronCores).

Problem: B=32 sequences, one new token each; KV cache [65536, 8, 128] f32
paged with PAGE=16; 32 query heads, 8 KV heads (GQA group 4), D=128.

Sharding: each core owns 2 KV heads x 16 sequences (batch split in two
interleaved-by-length halves for balance). Per core there are 32 "units"
(seq, head), each contributing 4 query rows -> 128 partition rows.

The kernel is DMA-bound (must stream the whole valid KV context), so all
packed device data is bf16: K/V/q are cast on the host during the pack.
rel-err vs the f32 reference is ~3e-3 (gate 2e-2).

Device pipeline per core:
  qdiag: memset [128, 4224] + one strided DVE copy turns the compact
        [128, 128] q tile into the staggered block-diagonal layout
        (unit u's 4 q columns at cols 132u..132u+4, so the lhsT slice
        [128u, 128u+128) has them at local cols 4u..4u+4).
  mm1:  scores[4u+g, j] = q_u . K_u[j] -- block-diagonal accumulation of
        bf16 matmuls (N=512) into one scores PSUM tile; K groups ride
        both HWDGE rings alternately.
  softmax: exp (ACT, scale=1/sqrt(D), accum_out -> per-chunk row sums)
        writes bf16; pad columns hold K=0 so exp=1 there and the host
        subtracts the pad count from the sum (no mask tensor at all).
  pT:   PE transposes of the bf16 exp tile (chunks of 128 positions).
  mm2:  o_u += pT_chunk(4 cols, stationary) @ V_chunk[128, 128] bf16
        (per-head moving slices; no discarded half).
  host: o = o_rows / (sum - npad), un-permute.

The program is value-specialized on the sorted per-unit chunk counts
(from context_lens); compiled programs are cached per signature.
"""

import numpy as np
import ml_dtypes

B = 32
KV_LEN = 2048
PAGE = 16
PAGES = KV_LEN // PAGE
H_Q = 32
H_KV = 8
D = 128
CH = 128                 # slot chunk (matmul contraction tile)
NCORES = 8
P = 128
SPC = 16                 # seqs per core
UPC = 32                 # units (seq, head) per core
SCALE = np.float32(1.0 / np.sqrt(D))
QW = 132                 # staggered block-diag column period for qT

_PROGRAM_CACHE = {}


def _k_groups(vps, max_cols):
    """Greedy-pack consecutive units into DMA groups of <=max_cols."""
    groups, cur, cols = [], [], 0
    for i, c in enumerate(vps):
        if cur and cols + c > max_cols:
            groups.append(cur)
            cur, cols = [], 0
        cur.append(i)
        cols += c
    if cur:
        groups.append(cur)
    return groups


def _build_program(ntU):
    """Build + compile the per-core program. ntU = per-seq-slot chunk counts
    (descending, len 16); unit u=2s+h has ntU[s] chunks."""
    import concourse.bacc as bacc
    import concourse.mybir as mybir
    import concourse.tile as tile
    from concourse.masks import make_identity
    from concourse.tile import add_dep_helper

    f32 = mybir.dt.float32
    bf16 = mybir.dt.bfloat16

    vpU = [ntU[u // 2] * CH for u in range(UPC)]          # per-unit K cols
    k_off = np.zeros(UPC + 1, np.int64)
    k_off[1:] = np.cumsum(vpU)
    TOTK = int(k_off[-1])
    v_cols = [ntU[s] * 2 * CH for s in range(SPC)]         # per-seq V cols
    v_off = np.zeros(SPC + 1, np.int64)
    v_off[1:] = np.cumsum(v_cols)
    assert int(v_off[-1]) == TOTK
    maxpad = vpU[0]
    maxnt = ntU[0]

    kgroups = _k_groups(vpU, 8192)
    vgroups = _k_groups(v_cols, 16384)

    nc = bacc.Bacc(
        "TRN2",
        target_bir_lowering=False,
        debug=False,
        enable_asserts=False,
        num_devices=NCORES,
    )
    qc_d = nc.dram_tensor("qc", [P, UPC * 4], bf16, kind="ExternalInput").ap()
    kT_d = nc.dram_tensor("kT", [P, TOTK], bf16, kind="ExternalInput").ap()
    vt_d = nc.dram_tensor("vt", [P, TOTK], bf16, kind="ExternalInput").ap()
    o_d = nc.dram_tensor("o", [P, P], f32, kind="ExternalOutput").ap()
    s_d = nc.dram_tensor("s", [P, maxnt], f32, kind="ExternalOutput").ap()

    with tile.TileContext(nc) as tc:
        with (
            tc.tile_pool(name="const", bufs=1) as constp,
            tc.tile_pool(name="kpool", bufs=3) as kpool,
            tc.tile_pool(name="vpool", bufs=len(vgroups)) as vpool,
            tc.tile_pool(name="sm", bufs=1) as smp,
            tc.tile_pool(name="tp", bufs=2, space="PSUM") as tpp,
            tc.tile_pool(name="ps_scores", bufs=1, space="PSUM") as pssc,
            tc.tile_pool(name="ps_o", bufs=2, space="PSUM") as pso,
        ):
            # Compact q load (32 KB) + on-device expansion to the staggered
            # block-diagonal stationary layout.
            qc_s = constp.tile([P, UPC * 4], bf16)
            nc.sync.dma_start(qc_s[:], qc_d[:])
            qT_s = constp.tile([P, QW * UPC], bf16)
            nc.vector.memset(qT_s[:], 0.0)
            qT3 = qT_s[:].rearrange("p (u c) -> p u c", u=UPC)
            qc3 = qc_s[:].rearrange("p (u g) -> p u g", u=UPC)
            nc.vector.tensor_copy(qT3[:, :, 0:4], qc3[:, :, :])
            ident = constp.tile([P, P], bf16)
            make_identity(nc, ident[:])

            # PE warm-up: back-to-back matmuls while the first K groups
            # stream in, so the HAM clock gate opens (1.2->2.4 GHz) before
            # real work starts and stays open through phase 1.
            warm_ps = pso.tile([4, D], f32, space="PSUM", tag="o")
            for w in range(12):
                nc.tensor.matmul(
                    warm_ps[:, :], ident[:, :4], ident[:, :],
                    start=True, stop=True,
                )

            e_t = smp.tile([P, maxpad], bf16)
            s_parts = smp.tile([P, maxnt], f32)
            scores_ps = pssc.tile([P, maxpad], f32, space="PSUM")

            # ---- phase 1: grouped K loads + block-diagonal mm1 ----
            # K groups alternate across the two HWDGE rings so both rings
            # stream from the start; V groups (phase 2's data) alternate the
            # opposite way and fill the vpool behind the K stream.
            first_mm = {}
            for gi, grp in enumerate(kgroups):
                g0, g1 = int(k_off[grp[0]]), int(k_off[grp[-1] + 1])
                kt = kpool.tile([P, 8192], bf16, tag="k")
                keng = nc.sync if gi % 2 == 0 else nc.scalar
                keng.dma_start(kt[:, :g1 - g0], kT_d[:, g0:g1])
                for u in grp:
                    vps = vpU[u]
                    uoff = int(k_off[u]) - g0
                    for r0 in range(0, vps, 512):
                        n = min(512, vps - r0)
                        bank = r0 // 512
                        mm = nc.tensor.matmul(
                            scores_ps[:, r0:r0 + n],
                            qT_s[:, P * u:P * u + P],
                            kt[:, uoff + r0:uoff + r0 + n],
                            start=(u == 0), stop=False,
                            skip_group_check=True,
                        )
                        if u == 0:
                            first_mm[bank] = mm
                        else:
                            add_dep_helper(
                                mm.ins, first_mm[bank].ins,
                                reason="pending-zero: unit-0 mm1 first")

            # ---- softmax + transposes, pipelined per chunk, high-c first --
            # High column chunks are written by few (long) units only, so
            # they complete while mm1 for later units is still streaming;
            # exp/transpose on them overlaps the tail of phase 1. Pad
            # columns hold score 0 -> exp 1; the host subtracts the count.
            pT = smp.tile([P, maxnt * CH], bf16)
            for c in range(maxnt - 1, -1, -1):
                cs = slice(c * CH, (c + 1) * CH)
                nc.scalar.activation(e_t[:, cs], scores_ps[:, cs],
                                     mybir.ActivationFunctionType.Exp,
                                     scale=float(SCALE),
                                     accum_out=s_parts[:, c:c + 1])
                tp = tpp.tile([P, P], bf16, space="PSUM", tag="tp")
                nc.tensor.transpose(tp[:], e_t[:, cs], ident[:])
                nc.vector.tensor_copy(pT[:, cs], tp[:])

            # ---- phase 2: grouped V loads + p-stationary mm2 ----
            # moving operand = per-head V chunk [128, 128] bf16.
            o_sb = smp.tile([4, UPC * D], f32)
            o_d3 = o_d.rearrange("(u g) d -> g u d", g=4)
            o_sb3 = o_sb[:].rearrange("g (u d) -> g u d", u=UPC)
            for gi, grp in enumerate(vgroups):
                g0, g1 = int(v_off[grp[0]]), int(v_off[grp[-1] + 1])
                vt = vpool.tile([P, 16384], bf16, tag="v")
                veng = nc.scalar if gi % 2 == 0 else nc.sync
                veng.dma_start(vt[:, :g1 - g0], vt_d[:, g0:g1])
                for s in grp:
                    soff = int(v_off[s]) - g0
                    for h in (0, 1):
                        u = 2 * s + h
                        o_ps = pso.tile([4, D], f32, space="PSUM", tag="o")
                        # descending c: high pT chunks are ready first
                        for ci, c in enumerate(range(ntU[s] - 1, -1, -1)):
                            nc.tensor.matmul(
                                o_ps[:, :],
                                pT[:, c * CH + 4 * u:c * CH + 4 * u + 4],
                                vt[:, soff + (2 * c + h) * D:
                                   soff + (2 * c + h) * D + D],
                                start=(ci == 0), stop=(c == 0),
                            )
                        nc.vector.tensor_copy(
                            o_sb[:, D * u:D * (u + 1)], o_ps[:, :])
            # single tail flush per ring (mid-stream flushes would block
            # later V loads behind them in the FIFO HWDGE rings)
            nc.sync.dma_start(o_d3[:, :, :], o_sb3[:, :, :])
            nc.scalar.dma_start(s_d[:], s_parts[:])

    nc.compile()
    return nc


def _host_prep(q, k, v, k_cache, v_cache, slot_mapping, block_tables,
               context_lens):
    """Build per-core packed inputs. Returns (in_maps, perm, ntU)."""
    ctx = np.clip(np.asarray(context_lens, np.int64), 1, KV_LEN)
    nt = (ctx + CH - 1) // CH
    perm = np.argsort(-nt, kind="stable")        # global length-sorted order
    nt_sorted = nt[perm]
    ntU = tuple(int(x) for x in nt_sorted[0::2])  # per-slot padded chunk count

    vpU = [ntU[u // 2] * CH for u in range(UPC)]
    k_off = np.zeros(UPC + 1, np.int64)
    k_off[1:] = np.cumsum(vpU)
    TOTK = int(k_off[-1])
    v_cols = [ntU[s] * 2 * CH for s in range(SPC)]
    v_off = np.zeros(SPC + 1, np.int64)
    v_off[1:] = np.cumsum(v_cols)

    bt = np.asarray(block_tables, np.int64)
    ident_bt = np.arange(B * PAGES, dtype=np.int64).reshape(B, PAGES)
    identity_layout = bt.shape == (B, PAGES) and np.array_equal(bt, ident_bt)
    slot_mapping = np.asarray(slot_mapping, np.int64)
    bf16 = ml_dtypes.bfloat16

    def gather_rows(cache_h, b, cta):
        """rows [0, cta) of seq b's context for one head slice, with the
        new-token scatter applied."""
        if identity_layout:
            rows = cache_h[b * KV_LEN:b * KV_LEN + cta]
            rel = slot_mapping - b * KV_LEN
            hit = np.nonzero((rel >= 0) & (rel < cta))[0]
            patch = (rel[hit], hit) if hit.size else None
        else:
            sids = (bt[b, :, None] * PAGE
                    + np.arange(PAGE)[None, :]).reshape(-1)[:cta]
            rows = cache_h[sids]
            pos, src = np.nonzero(sids[:, None] == slot_mapping[None, :])
            patch = (pos, src) if pos.size else None
        return rows, patch

    in_maps = []
    for m in range(NCORES):
        hb, hp = m // 4, m % 4
        qc = np.zeros((P, UPC * 4), bf16)
        kT_packed = np.zeros((P, TOTK), bf16)
        vt_packed = np.zeros((P, TOTK), bf16)
        for s in range(SPC):
            b = int(perm[2 * s + hb])
            ntu = ntU[s]
            cta = int(ctx[b])                 # actual cols; rest stays zero
            for h in (0, 1):
                u = 2 * s + h
                head = 2 * hp + h
                qc[:, 4 * u:4 * u + 4] = q[b, 4 * head:4 * head + 4, :].T

                krows, patch = gather_rows(k_cache[:, head, :], b, cta)
                if patch is not None:
                    krows = krows.copy()
                    krows[patch[0]] = k[patch[1], head, :]
                o0 = int(k_off[u])
                kT_packed[:, o0:o0 + cta] = krows.T

                vrows, patch = gather_rows(v_cache[:, head, :], b, cta)
                if patch is not None:
                    vrows = vrows.copy()
                    vrows[patch[0]] = v[patch[1], head, :]
                # vt layout per seq: [jj, c, h, d]
                vo = int(v_off[s])
                vt3 = vt_packed[:, vo:vo + ntu * 2 * D].reshape(P, ntu, 2, D)
                vfull = np.zeros((ntu * CH, D), np.float32)
                vfull[:cta] = vrows
                vt3[:, :, h, :] = vfull.reshape(ntu, CH, D).transpose(1, 0, 2)

        in_maps.append(dict(qc=qc, kT=kT_packed, vt=vt_packed))

    return in_maps, perm, ntU


def kernel(q, k, v, k_cache, v_cache, slot_mapping, block_tables,
           context_lens, _trace=False):
    from concourse import bass_utils

    q = np.asarray(q, np.float32)
    k = np.asarray(k, np.float32)
    v = np.asarray(v, np.float32)
    k_cache = np.asarray(k_cache, np.float32)
    v_cache = np.asarray(v_cache, np.float32)

    in_maps, perm, ntU = _host_prep(
        q, k, v, k_cache, v_cache, slot_mapping, block_tables, context_lens)

    if ntU not in _PROGRAM_CACHE:
        _PROGRAM_CACHE[ntU] = _build_program(ntU)
    nc = _PROGRAM_CACHE[ntU]

    res = bass_utils.run_bass_kernel_spmd(
        nc, in_maps, core_ids=list(range(NCORES)), trace=_trace)

    ctx = np.clip(np.asarray(context_lens, np.int64), 1, KV_LEN)
    maxpad = ntU[0] * CH
    o = np.empty((B, H_Q, D), np.float32)
    for m in range(NCORES):
        hb, hp = m // 4, m % 4
        om = res.results[m]["o"]
        ssum = res.results[m]["s"].astype(np.float64).sum(axis=1)
        for s in range(SPC):
            b = int(perm[2 * s + hb])
            denom = ssum[4 * (2 * s):4 * (2 * s) + 8] - (maxpad - int(ctx[b]))
            for h in (0, 1):
                u = 2 * s + h
                head = 2 * hp + h
                o[b, 4 * head:4 * head + 4, :] = (
                    om[4 * u:4 * u + 4, :]
                    / denom[4 * h:4 * h + 4, None]).astype(np.float32)
    if _trace:
        kernel._last_result = res
    return o


# revision 11
# speedup vs baseline: 1.0147x; 1.0147x over previous
"""Paged-attention GQA decode kernel for Trainium2 (8 NeuronCores).

Problem: B=32 sequences, one new token each; KV cache [65536, 8, 128] f32
paged with PAGE=16; 32 query heads, 8 KV heads (GQA group 4), D=128.

Sharding: each core owns 2 KV heads x 16 sequences (batch split in two
interleaved-by-length halves for balance). Per core there are 32 "units"
(seq, head), each contributing 4 query rows -> 128 partition rows.

The kernel is DMA-bound (must stream the whole valid KV context), so all
packed device data is bf16: K/V/q are cast on the host during the pack.
rel-err vs the f32 reference is ~3e-3 (gate 2e-2).

Device pipeline per core:
  qdiag: memset [128, 4224] + one strided DVE copy turns the compact
        [128, 128] q tile into the staggered block-diagonal layout
        (unit u's 4 q columns at cols 132u..132u+4, so the lhsT slice
        [128u, 128u+128) has them at local cols 4u..4u+4).
  mm1:  scores[4u+g, j] = q_u . K_u[j] -- block-diagonal accumulation of
        bf16 matmuls (N=512) into one scores PSUM tile; K groups ride
        both HWDGE rings alternately.
  softmax: exp (ACT, scale=1/sqrt(D), accum_out -> per-chunk row sums)
        writes bf16; pad columns hold K=0 so exp=1 there and the host
        subtracts the pad count from the sum (no mask tensor at all).
  pT:   PE transposes of the bf16 exp tile (chunks of 128 positions).
  mm2:  o_u += pT_chunk(4 cols, stationary) @ V_chunk[128, 128] bf16
        (per-head moving slices; no discarded half).
  host: o = o_rows / (sum - npad), un-permute.

The program is value-specialized on the sorted per-unit chunk counts
(from context_lens); compiled programs are cached per signature.
"""

import numpy as np
import ml_dtypes

B = 32
KV_LEN = 2048
PAGE = 16
PAGES = KV_LEN // PAGE
H_Q = 32
H_KV = 8
D = 128
CH = 128                 # slot chunk (matmul contraction tile)
NCORES = 8
P = 128
SPC = 16                 # seqs per core
UPC = 32                 # units (seq, head) per core
SCALE = np.float32(1.0 / np.sqrt(D))
QW = 132                 # staggered block-diag column period for qT

_PROGRAM_CACHE = {}


def _k_groups(vps, max_cols):
    """Greedy-pack consecutive units into DMA groups of <=max_cols."""
    groups, cur, cols = [], [], 0
    for i, c in enumerate(vps):
        if cur and cols + c > max_cols:
            groups.append(cur)
            cur, cols = [], 0
        cur.append(i)
        cols += c
    if cur:
        groups.append(cur)
    return groups


def _build_program(ntU):
    """Build + compile the per-core program. ntU = per-seq-slot chunk counts
    (descending, len 16); unit u=2s+h has ntU[s] chunks."""
    import concourse.bacc as bacc
    import concourse.mybir as mybir
    import concourse.tile as tile
    from concourse.masks import make_identity
    from concourse.tile import add_dep_helper

    f32 = mybir.dt.float32
    bf16 = mybir.dt.bfloat16

    vpU = [ntU[u // 2] * CH for u in range(UPC)]          # per-unit K cols
    k_off = np.zeros(UPC + 1, np.int64)
    k_off[1:] = np.cumsum(vpU)
    TOTK = int(k_off[-1])
    v_cols = [ntU[s] * 2 * CH for s in range(SPC)]         # per-seq V cols
    v_off = np.zeros(SPC + 1, np.int64)
    v_off[1:] = np.cumsum(v_cols)
    assert int(v_off[-1]) == TOTK
    maxpad = vpU[0]
    maxnt = ntU[0]

    kgroups = _k_groups(vpU, 8192)
    vgroups = _k_groups(v_cols, 16384)

    nc = bacc.Bacc(
        "TRN2",
        target_bir_lowering=False,
        debug=False,
        enable_asserts=False,
        num_devices=NCORES,
    )
    qc_d = nc.dram_tensor("qc", [P, UPC * 4], bf16, kind="ExternalInput").ap()
    kT_d = nc.dram_tensor("kT", [P, TOTK], bf16, kind="ExternalInput").ap()
    vt_d = nc.dram_tensor("vt", [P, TOTK], bf16, kind="ExternalInput").ap()
    o_d = nc.dram_tensor("o", [P, P], f32, kind="ExternalOutput").ap()
    s_d = nc.dram_tensor("s", [P, maxnt], f32, kind="ExternalOutput").ap()

    with tile.TileContext(nc) as tc:
        with (
            tc.tile_pool(name="const", bufs=1) as constp,
            tc.tile_pool(name="kpool", bufs=min(4, len(kgroups))) as kpool,
            tc.tile_pool(name="vpool", bufs=len(vgroups)) as vpool,
            tc.tile_pool(name="sm", bufs=1) as smp,
            tc.tile_pool(name="tp", bufs=2, space="PSUM") as tpp,
            tc.tile_pool(name="ps_scores", bufs=1, space="PSUM") as pssc,
            tc.tile_pool(name="ps_o", bufs=2, space="PSUM") as pso,
        ):
            # Compact q load (32 KB) + on-device expansion to the staggered
            # block-diagonal stationary layout.
            qc_s = constp.tile([P, UPC * 4], bf16)
            nc.sync.dma_start(qc_s[:], qc_d[:])
            qT_s = constp.tile([P, QW * UPC], bf16)
            nc.vector.memset(qT_s[:], 0.0)
            qT3 = qT_s[:].rearrange("p (u c) -> p u c", u=UPC)
            qc3 = qc_s[:].rearrange("p (u g) -> p u g", u=UPC)
            nc.vector.tensor_copy(qT3[:, :, 0:4], qc3[:, :, :])
            ident = constp.tile([P, P], bf16)
            make_identity(nc, ident[:])

            # PE warm-up: back-to-back matmuls while the first K groups
            # stream in, so the HAM clock gate opens (1.2->2.4 GHz) before
            # real work starts and stays open through phase 1.
            warm_ps = pso.tile([4, D], f32, space="PSUM", tag="o")
            for w in range(12):
                nc.tensor.matmul(
                    warm_ps[:, :], ident[:, :4], ident[:, :],
                    start=True, stop=True,
                )

            e_t = smp.tile([P, maxpad], bf16)
            s_parts = smp.tile([P, maxnt], f32)
            scores_ps = pssc.tile([P, maxpad], f32, space="PSUM")

            # ---- DMA issue, all upfront: K first on both rings (K gates
            # scores -> exp -> pT -> every unit's mm2 tail), then V groups
            # (consumed later, fully SBUF-resident so nothing downstream
            # ever blocks the rings).
            kts = []
            for gi, grp in enumerate(kgroups):
                g0, g1 = int(k_off[grp[0]]), int(k_off[grp[-1] + 1])
                kt = kpool.tile([P, 8192], bf16, tag="k")
                keng = nc.sync if gi % 2 == 0 else nc.scalar
                keng.dma_start(kt[:, :g1 - g0], kT_d[:, g0:g1])
                kts.append(kt)
            vts = []
            for gi, grp in enumerate(vgroups):
                g0, g1 = int(v_off[grp[0]]), int(v_off[grp[-1] + 1])
                vt = vpool.tile([P, 16384], bf16, tag="v")
                veng = nc.scalar if gi % 2 == 0 else nc.sync
                veng.dma_start(vt[:, :g1 - g0], vt_d[:, g0:g1])
                vts.append(vt)

            # ---- phase 1: block-diagonal mm1 over the K stream ----
            first_mm = {}
            for gi, grp in enumerate(kgroups):
                g0 = int(k_off[grp[0]])
                kt = kts[gi]
                for u in grp:
                    vps = vpU[u]
                    uoff = int(k_off[u]) - g0
                    for r0 in range(0, vps, 512):
                        n = min(512, vps - r0)
                        bank = r0 // 512
                        mm = nc.tensor.matmul(
                            scores_ps[:, r0:r0 + n],
                            qT_s[:, P * u:P * u + P],
                            kt[:, uoff + r0:uoff + r0 + n],
                            start=(u == 0), stop=False,
                            skip_group_check=True,
                        )
                        if u == 0:
                            first_mm[bank] = mm
                        else:
                            add_dep_helper(
                                mm.ins, first_mm[bank].ins,
                                reason="pending-zero: unit-0 mm1 first")

            # ---- softmax + transposes, pipelined per chunk, high-c first --
            # High column chunks are written by few (long) units only, so
            # they complete while mm1 for later units is still streaming;
            # exp/transpose on them overlaps the tail of phase 1. Pad
            # columns hold score 0 -> exp 1; the host subtracts the count.
            pT = smp.tile([P, maxnt * CH], bf16)
            for c in range(maxnt - 1, -1, -1):
                cs = slice(c * CH, (c + 1) * CH)
                nc.scalar.activation(e_t[:, cs], scores_ps[:, cs],
                                     mybir.ActivationFunctionType.Exp,
                                     scale=float(SCALE),
                                     accum_out=s_parts[:, c:c + 1])
                tp = tpp.tile([P, P], bf16, space="PSUM", tag="tp")
                nc.tensor.transpose(tp[:], e_t[:, cs], ident[:])
                nc.vector.tensor_copy(pT[:, cs], tp[:])

            # ---- phase 2: grouped V loads + p-stationary mm2 ----
            # moving operand = per-head V chunk [128, 128] bf16.
            o_sb = smp.tile([4, UPC * D], f32)
            o_d3 = o_d.rearrange("(u g) d -> g u d", g=4)
            o_sb3 = o_sb[:].rearrange("g (u d) -> g u d", u=UPC)
            for gi, grp in enumerate(vgroups):
                g0 = int(v_off[grp[0]])
                vt = vts[gi]
                for s in grp:
                    soff = int(v_off[s]) - g0
                    for h in (0, 1):
                        u = 2 * s + h
                        o_ps = pso.tile([4, D], f32, space="PSUM", tag="o")
                        # descending c: high pT chunks are ready first
                        for ci, c in enumerate(range(ntU[s] - 1, -1, -1)):
                            nc.tensor.matmul(
                                o_ps[:, :],
                                pT[:, c * CH + 4 * u:c * CH + 4 * u + 4],
                                vt[:, soff + (2 * c + h) * D:
                                   soff + (2 * c + h) * D + D],
                                start=(ci == 0), stop=(c == 0),
                            )
                        nc.vector.tensor_copy(
                            o_sb[:, D * u:D * (u + 1)], o_ps[:, :])
            # single tail flush per ring (mid-stream flushes would block
            # later V loads behind them in the FIFO HWDGE rings)
            nc.sync.dma_start(o_d3[:, :, :], o_sb3[:, :, :])
            nc.scalar.dma_start(s_d[:], s_parts[:])

    nc.compile()
    return nc


def _host_prep(q, k, v, k_cache, v_cache, slot_mapping, block_tables,
               context_lens):
    """Build per-core packed inputs. Returns (in_maps, perm, ntU)."""
    ctx = np.clip(np.asarray(context_lens, np.int64), 1, KV_LEN)
    nt = (ctx + CH - 1) // CH
    perm = np.argsort(-nt, kind="stable")        # global length-sorted order
    nt_sorted = nt[perm]
    ntU = tuple(int(x) for x in nt_sorted[0::2])  # per-slot padded chunk count

    vpU = [ntU[u // 2] * CH for u in range(UPC)]
    k_off = np.zeros(UPC + 1, np.int64)
    k_off[1:] = np.cumsum(vpU)
    TOTK = int(k_off[-1])
    v_cols = [ntU[s] * 2 * CH for s in range(SPC)]
    v_off = np.zeros(SPC + 1, np.int64)
    v_off[1:] = np.cumsum(v_cols)

    bt = np.asarray(block_tables, np.int64)
    ident_bt = np.arange(B * PAGES, dtype=np.int64).reshape(B, PAGES)
    identity_layout = bt.shape == (B, PAGES) and np.array_equal(bt, ident_bt)
    slot_mapping = np.asarray(slot_mapping, np.int64)
    bf16 = ml_dtypes.bfloat16

    def gather_rows(cache_h, b, cta):
        """rows [0, cta) of seq b's context for one head slice, with the
        new-token scatter applied."""
        if identity_layout:
            rows = cache_h[b * KV_LEN:b * KV_LEN + cta]
            rel = slot_mapping - b * KV_LEN
            hit = np.nonzero((rel >= 0) & (rel < cta))[0]
            patch = (rel[hit], hit) if hit.size else None
        else:
            sids = (bt[b, :, None] * PAGE
                    + np.arange(PAGE)[None, :]).reshape(-1)[:cta]
            rows = cache_h[sids]
            pos, src = np.nonzero(sids[:, None] == slot_mapping[None, :])
            patch = (pos, src) if pos.size else None
        return rows, patch

    in_maps = []
    for m in range(NCORES):
        hb, hp = m // 4, m % 4
        qc = np.zeros((P, UPC * 4), bf16)
        kT_packed = np.zeros((P, TOTK), bf16)
        vt_packed = np.zeros((P, TOTK), bf16)
        for s in range(SPC):
            b = int(perm[2 * s + hb])
            ntu = ntU[s]
            cta = int(ctx[b])                 # actual cols; rest stays zero
            for h in (0, 1):
                u = 2 * s + h
                head = 2 * hp + h
                qc[:, 4 * u:4 * u + 4] = q[b, 4 * head:4 * head + 4, :].T

                krows, patch = gather_rows(k_cache[:, head, :], b, cta)
                if patch is not None:
                    krows = krows.copy()
                    krows[patch[0]] = k[patch[1], head, :]
                o0 = int(k_off[u])
                kT_packed[:, o0:o0 + cta] = krows.T

                vrows, patch = gather_rows(v_cache[:, head, :], b, cta)
                if patch is not None:
                    vrows = vrows.copy()
                    vrows[patch[0]] = v[patch[1], head, :]
                # vt layout per seq: [jj, c, h, d]
                vo = int(v_off[s])
                vt3 = vt_packed[:, vo:vo + ntu * 2 * D].reshape(P, ntu, 2, D)
                vfull = np.zeros((ntu * CH, D), np.float32)
                vfull[:cta] = vrows
                vt3[:, :, h, :] = vfull.reshape(ntu, CH, D).transpose(1, 0, 2)

        in_maps.append(dict(qc=qc, kT=kT_packed, vt=vt_packed))

    return in_maps, perm, ntU


def kernel(q, k, v, k_cache, v_cache, slot_mapping, block_tables,
           context_lens, _trace=False):
    from concourse import bass_utils

    q = np.asarray(q, np.float32)
    k = np.asarray(k, np.float32)
    v = np.asarray(v, np.float32)
    k_cache = np.asarray(k_cache, np.float32)
    v_cache = np.asarray(v_cache, np.float32)

    in_maps, perm, ntU = _host_prep(
        q, k, v, k_cache, v_cache, slot_mapping, block_tables, context_lens)

    if ntU not in _PROGRAM_CACHE:
        _PROGRAM_CACHE[ntU] = _build_program(ntU)
    nc = _PROGRAM_CACHE[ntU]

    res = bass_utils.run_bass_kernel_spmd(
        nc, in_maps, core_ids=list(range(NCORES)), trace=_trace)

    ctx = np.clip(np.asarray(context_lens, np.int64), 1, KV_LEN)
    maxpad = ntU[0] * CH
    o = np.empty((B, H_Q, D), np.float32)
    for m in range(NCORES):
        hb, hp = m // 4, m % 4
        om = res.results[m]["o"]
        ssum = res.results[m]["s"].astype(np.float64).sum(axis=1)
        for s in range(SPC):
            b = int(perm[2 * s + hb])
            denom = ssum[4 * (2 * s):4 * (2 * s) + 8] - (maxpad - int(ctx[b]))
            for h in (0, 1):
                u = 2 * s + h
                head = 2 * hp + h
                o[b, 4 * head:4 * head + 4, :] = (
                    om[4 * u:4 * u + 4, :]
                    / denom[4 * h:4 * h + 4, None]).astype(np.float32)
    if _trace:
        kernel._last_result = res
    return o


# revision 21
# speedup vs baseline: 1.2218x; 1.2040x over previous
"""Paged-attention GQA decode kernel for Trainium2 (8 NeuronCores).

Problem: B=32 sequences, one new token each; KV cache [65536, 8, 128] f32
paged with PAGE=16; 32 query heads, 8 KV heads (GQA group 4), D=128.

Sharding: each core owns 2 KV heads x 16 sequences (batch split in two
interleaved-by-length halves for balance). Per core there are 32 "units"
(seq, head), each contributing 4 query rows -> 128 partition rows.

The kernel is DMA-bound (must stream the whole valid KV context), so all
packed device data is bf16: K/V/q are cast on the host during the pack.
rel-err vs the f32 reference is ~3e-3 (gate 2e-2).

Device pipeline per core:
  qdiag: memset [128, 4224] + one strided DVE copy turns the compact
        [128, 128] q tile into the staggered block-diagonal layout
        (unit u's 4 q columns at cols 132u..132u+4, so the lhsT slice
        [128u, 128u+128) has them at local cols 4u..4u+4).
  mm1:  scores[4u+g, j] = q_u . K_u[j] -- block-diagonal accumulation of
        bf16 matmuls (N=512) into one scores PSUM tile; K groups ride
        both HWDGE rings alternately.
  softmax: exp (ACT, scale=1/sqrt(D), accum_out -> per-chunk row sums)
        writes bf16; pad columns hold K=0 so exp=1 there and the host
        subtracts the pad count from the sum (no mask tensor at all).
  pT:   PE transposes of the bf16 exp tile (chunks of 128 positions).
  mm2:  o_u += pT_chunk(4 cols, stationary) @ V_chunk[128, 128] bf16
        (per-head moving slices; no discarded half).
  host: o = o_rows / (sum - npad), un-permute.

The program is value-specialized on the sorted per-unit chunk counts
(from context_lens); compiled programs are cached per signature.
"""

import numpy as np
import ml_dtypes

B = 32
KV_LEN = 2048
PAGE = 16
PAGES = KV_LEN // PAGE
H_Q = 32
H_KV = 8
D = 128
CH = 128                 # slot chunk (matmul contraction tile)
NCORES = 8
P = 128
SPC = 16                 # seqs per core
UPC = 32                 # units (seq, head) per core
SCALE = np.float32(1.0 / np.sqrt(D))
QW = 132                 # staggered block-diag column period for qT

_PROGRAM_CACHE = {}


def _k_groups(vps, max_cols):
    """Greedy-pack consecutive units into DMA groups of <=max_cols."""
    groups, cur, cols = [], [], 0
    for i, c in enumerate(vps):
        if cur and cols + c > max_cols:
            groups.append(cur)
            cur, cols = [], 0
        cur.append(i)
        cols += c
    if cur:
        groups.append(cur)
    return groups


def _build_program(ntU):
    """Build + compile the per-core program. ntU = per-seq-slot chunk counts
    (descending, len 16); unit u=2s+h has ntU[s] chunks."""
    import concourse.bacc as bacc
    import concourse.mybir as mybir
    import concourse.tile as tile
    from concourse.masks import make_identity
    from concourse.tile import add_dep_helper

    f32 = mybir.dt.float32
    bf16 = mybir.dt.bfloat16

    vpU = [ntU[u // 2] * CH for u in range(UPC)]          # per-unit K cols
    k_off = np.zeros(UPC + 1, np.int64)
    k_off[1:] = np.cumsum(vpU)
    TOTK = int(k_off[-1])
    v_cols = [ntU[s] * 2 * CH for s in range(SPC)]         # per-seq V cols
    v_off = np.zeros(SPC + 1, np.int64)
    v_off[1:] = np.cumsum(v_cols)
    assert int(v_off[-1]) == TOTK
    maxpad = vpU[0]
    maxnt = ntU[0]

    kgroups = _k_groups(vpU, 8192)
    vgroups = _k_groups(v_cols, 4096)
    # PSUM bank = 512 f32 cols = 4 chunks. Per bank: its last writer unit
    # (stop there -- start/stop have whole-bank HW side effects, so unit 0
    # opens each bank with one bank-wide N=512 matmul and the stop rides
    # the final write of the bank).
    nbanks = (maxnt + 3) // 4
    bank_stop = {}
    for b in range(nbanks):
        lu = max(2 * s + 1 for s in range(SPC) if ntU[s] > 4 * b)
        cstar = min(ntU[lu // 2], 4 * b + 4) - 1
        bank_stop[b] = (lu, cstar)

    nc = bacc.Bacc(
        "TRN2",
        target_bir_lowering=False,
        debug=False,
        enable_asserts=False,
        num_devices=NCORES,
    )
    qc_d = nc.dram_tensor("qc", [P, UPC * 4], bf16, kind="ExternalInput").ap()
    kT_d = nc.dram_tensor("kT", [P, TOTK], bf16, kind="ExternalInput").ap()
    vt_d = nc.dram_tensor("vt", [P, TOTK], bf16, kind="ExternalInput").ap()
    o_d = nc.dram_tensor("o", [P, P], f32, kind="ExternalOutput").ap()
    s_d = nc.dram_tensor("s", [P, maxnt], f32, kind="ExternalOutput").ap()

    with tile.TileContext(nc) as tc:
        with (
            tc.tile_pool(name="const", bufs=1) as constp,
            tc.tile_pool(name="kpool", bufs=len(kgroups)) as kpool,
            tc.tile_pool(name="vpool", bufs=len(vgroups)) as vpool,
            tc.tile_pool(name="sm", bufs=1) as smp,
            tc.tile_pool(name="tp", bufs=2, space="PSUM") as tpp,
            tc.tile_pool(name="ps_scores", bufs=1, space="PSUM") as pssc,
            tc.tile_pool(name="ps_o", bufs=2, space="PSUM") as pso,
        ):
            # Compact q load (32 KB) + on-device expansion to the staggered
            # block-diagonal stationary layout.
            qc_s = constp.tile([P, UPC * 4], bf16)
            nc.sync.dma_start(qc_s[:], qc_d[:])
            qT_s = constp.tile([P, QW * UPC], bf16)
            nc.vector.memset(qT_s[:], 0.0)
            qT3 = qT_s[:].rearrange("p (u c) -> p u c", u=UPC)
            qc3 = qc_s[:].rearrange("p (u g) -> p u g", u=UPC)
            nc.vector.tensor_copy(qT3[:, :, 0:4], qc3[:, :, :])
            ident = constp.tile([P, P], bf16)
            make_identity(nc, ident[:])

            s_parts = smp.tile([P, maxnt], f32)
            # per-bank scores tiles: separate dependency domains, so exp of
            # a high bank unblocks as soon as that bank's writers finish
            scores_b = [pssc.tile([P, 512], f32, space="PSUM", tag=f"sc{b}",
                                  name=f"scores{b}")
                        for b in range(nbanks)]

            # ---- DMA issue, all upfront: K first on both rings (K gates
            # scores -> exp -> pT -> every unit's mm2 tail), then V groups
            # (consumed later, fully SBUF-resident so nothing downstream
            # ever blocks the rings).
            kts = []
            for gi, grp in enumerate(kgroups):
                g0, g1 = int(k_off[grp[0]]), int(k_off[grp[-1] + 1])
                kt = kpool.tile([P, 8192], bf16, tag="k")
                keng = nc.sync if gi % 2 == 0 else nc.scalar
                keng.dma_start(kt[:, :g1 - g0], kT_d[:, g0:g1])
                kts.append(kt)
            vts = []
            for gi, grp in enumerate(vgroups):
                g0, g1 = int(v_off[grp[0]]), int(v_off[grp[-1] + 1])
                vt = vpool.tile([P, 4096], bf16, tag="v")
                veng = nc.scalar if gi % 2 == 0 else nc.sync
                veng.dma_start(vt[:, :g1 - g0], vt_d[:, g0:g1])
                vts.append(vt)

            # ---- phase 1: block-diagonal mm1 over the K stream ----
            # Unit 0 opens each bank with one bank-wide start=True matmul
            # (start clears has_written for the WHOLE bank, so it must be
            # bank-aligned and fired exactly once per bank). Other units
            # accumulate at chunk (128-col) granularity; the bank's very
            # last write carries stop=True.
            first_mm = {}
            for gi, grp in enumerate(kgroups):
                g0 = int(k_off[grp[0]])
                kt = kts[gi]
                for u in grp:
                    vps = vpU[u]
                    uoff = int(k_off[u]) - g0
                    step = 512 if u == 0 else CH
                    for r0 in range(0, vps, step):
                        n = min(step, vps - r0)
                        b, c = r0 // 512, r0 // CH
                        mm = nc.tensor.matmul(
                            scores_b[b][:, r0 - 512 * b:r0 - 512 * b + n],
                            qT_s[:, P * u:P * u + P],
                            kt[:, uoff + r0:uoff + r0 + n],
                            start=(u == 0),
                            stop=(bank_stop[b] == (u, c)),
                            skip_group_check=True,
                        )
                        if u == 0:
                            first_mm[b] = mm
                        else:
                            add_dep_helper(
                                mm.ins, first_mm[b].ins,
                                reason="pending-zero: unit-0 mm1 first")

            # ---- softmax + transposes, pipelined per chunk, high-c first --
            # High banks are closed by the K stream early, so their exp /
            # transpose / copy overlap the tail of phase 1. Per-chunk e/pT
            # tiles keep the chunks in independent dependency domains. Pad
            # columns hold score 0 -> exp 1; the host subtracts the count.
            e_c = {}
            pT_c = {}
            for c in range(maxnt - 1, -1, -1):
                b, bo = c // 4, (c % 4) * CH
                e_c[c] = smp.tile([P, CH], bf16, tag=f"e{c}", name=f"e{c}")
                nc.scalar.activation(e_c[c][:], scores_b[b][:, bo:bo + CH],
                                     mybir.ActivationFunctionType.Exp,
                                     scale=float(SCALE),
                                     accum_out=s_parts[:, c:c + 1])
                tp = tpp.tile([P, P], bf16, space="PSUM", tag="tp")
                nc.tensor.transpose(tp[:], e_c[c][:], ident[:])
                pT_c[c] = smp.tile([P, CH], bf16, tag=f"pT{c}", name=f"pT{c}")
                nc.vector.tensor_copy(pT_c[c][:], tp[:])

            # ---- phase 2: grouped V loads + p-stationary mm2 ----
            # moving operand = per-head V chunk [128, 128] bf16.
            o_sb = smp.tile([4, UPC * D], f32)
            o_d3 = o_d.rearrange("(u g) d -> g u d", g=4)
            o_sb3 = o_sb[:].rearrange("g (u d) -> g u d", u=UPC)
            for gi, grp in enumerate(vgroups):
                g0 = int(v_off[grp[0]])
                vt = vts[gi]
                for s in grp:
                    soff = int(v_off[s]) - g0
                    for h in (0, 1):
                        u = 2 * s + h
                        o_ps = pso.tile([4, D], f32, space="PSUM", tag="o")
                        # descending c: high pT chunks are ready first
                        for ci, c in enumerate(range(ntU[s] - 1, -1, -1)):
                            nc.tensor.matmul(
                                o_ps[:, :],
                                pT_c[c][:, 4 * u:4 * u + 4],
                                vt[:, soff + (2 * c + h) * D:
                                   soff + (2 * c + h) * D + D],
                                start=(ci == 0), stop=(c == 0),
                            )
                        nc.vector.tensor_copy(
                            o_sb[:, D * u:D * (u + 1)], o_ps[:, :])
            # single tail flush per ring (mid-stream flushes would block
            # later V loads behind them in the FIFO HWDGE rings)
            nc.sync.dma_start(o_d3[:, :, :], o_sb3[:, :, :])
            nc.scalar.dma_start(s_d[:], s_parts[:])

    nc.compile()
    return nc


def _host_prep(q, k, v, k_cache, v_cache, slot_mapping, block_tables,
               context_lens):
    """Build per-core packed inputs. Returns (in_maps, perm, ntU)."""
    ctx = np.clip(np.asarray(context_lens, np.int64), 1, KV_LEN)
    nt = (ctx + CH - 1) // CH
    perm = np.argsort(-nt, kind="stable")        # global length-sorted order
    nt_sorted = nt[perm]
    ntU = tuple(int(x) for x in nt_sorted[0::2])  # per-slot padded chunk count

    vpU = [ntU[u // 2] * CH for u in range(UPC)]
    k_off = np.zeros(UPC + 1, np.int64)
    k_off[1:] = np.cumsum(vpU)
    TOTK = int(k_off[-1])
    v_cols = [ntU[s] * 2 * CH for s in range(SPC)]
    v_off = np.zeros(SPC + 1, np.int64)
    v_off[1:] = np.cumsum(v_cols)

    bt = np.asarray(block_tables, np.int64)
    ident_bt = np.arange(B * PAGES, dtype=np.int64).reshape(B, PAGES)
    identity_layout = bt.shape == (B, PAGES) and np.array_equal(bt, ident_bt)
    slot_mapping = np.asarray(slot_mapping, np.int64)
    bf16 = ml_dtypes.bfloat16

    def gather_rows(cache_h, b, cta):
        """rows [0, cta) of seq b's context for one head slice, with the
        new-token scatter applied."""
        if identity_layout:
            rows = cache_h[b * KV_LEN:b * KV_LEN + cta]
            rel = slot_mapping - b * KV_LEN
            hit = np.nonzero((rel >= 0) & (rel < cta))[0]
            patch = (rel[hit], hit) if hit.size else None
        else:
            sids = (bt[b, :, None] * PAGE
                    + np.arange(PAGE)[None, :]).reshape(-1)[:cta]
            rows = cache_h[sids]
            pos, src = np.nonzero(sids[:, None] == slot_mapping[None, :])
            patch = (pos, src) if pos.size else None
        return rows, patch

    in_maps = []
    for m in range(NCORES):
        hb, hp = m // 4, m % 4
        qc = np.zeros((P, UPC * 4), bf16)
        kT_packed = np.zeros((P, TOTK), bf16)
        vt_packed = np.zeros((P, TOTK), bf16)
        for s in range(SPC):
            b = int(perm[2 * s + hb])
            ntu = ntU[s]
            cta = int(ctx[b])                 # actual cols; rest stays zero
            for h in (0, 1):
                u = 2 * s + h
                head = 2 * hp + h
                qc[:, 4 * u:4 * u + 4] = q[b, 4 * head:4 * head + 4, :].T

                krows, patch = gather_rows(k_cache[:, head, :], b, cta)
                if patch is not None:
                    krows = krows.copy()
                    krows[patch[0]] = k[patch[1], head, :]
                o0 = int(k_off[u])
                kT_packed[:, o0:o0 + cta] = krows.T

                vrows, patch = gather_rows(v_cache[:, head, :], b, cta)
                if patch is not None:
                    vrows = vrows.copy()
                    vrows[patch[0]] = v[patch[1], head, :]
                # vt layout per seq: [jj, c, h, d]
                vo = int(v_off[s])
                vt3 = vt_packed[:, vo:vo + ntu * 2 * D].reshape(P, ntu, 2, D)
                vfull = np.zeros((ntu * CH, D), np.float32)
                vfull[:cta] = vrows
                vt3[:, :, h, :] = vfull.reshape(ntu, CH, D).transpose(1, 0, 2)

        in_maps.append(dict(qc=qc, kT=kT_packed, vt=vt_packed))

    return in_maps, perm, ntU


def kernel(q, k, v, k_cache, v_cache, slot_mapping, block_tables,
           context_lens, _trace=False):
    from concourse import bass_utils

    q = np.asarray(q, np.float32)
    k = np.asarray(k, np.float32)
    v = np.asarray(v, np.float32)
    k_cache = np.asarray(k_cache, np.float32)
    v_cache = np.asarray(v_cache, np.float32)

    in_maps, perm, ntU = _host_prep(
        q, k, v, k_cache, v_cache, slot_mapping, block_tables, context_lens)

    if ntU not in _PROGRAM_CACHE:
        _PROGRAM_CACHE[ntU] = _build_program(ntU)
    nc = _PROGRAM_CACHE[ntU]

    res = bass_utils.run_bass_kernel_spmd(
        nc, in_maps, core_ids=list(range(NCORES)), trace=_trace)

    ctx = np.clip(np.asarray(context_lens, np.int64), 1, KV_LEN)
    maxpad = ntU[0] * CH
    o = np.empty((B, H_Q, D), np.float32)
    for m in range(NCORES):
        hb, hp = m // 4, m % 4
        om = res.results[m]["o"]
        ssum = res.results[m]["s"].astype(np.float64).sum(axis=1)
        for s in range(SPC):
            b = int(perm[2 * s + hb])
            denom = ssum[4 * (2 * s):4 * (2 * s) + 8] - (maxpad - int(ctx[b]))
            for h in (0, 1):
                u = 2 * s + h
                head = 2 * hp + h
                o[b, 4 * head:4 * head + 4, :] = (
                    om[4 * u:4 * u + 4, :]
                    / denom[4 * h:4 * h + 4, None]).astype(np.float32)
    if _trace:
        kernel._last_result = res
    return o
